# revision 1
# baseline (speedup 1.0000x reference)
"""GAT (3-layer, PPI-style) Bass/Tile kernel for 8 Trainium2 NeuronCores.

Strategy (graph/data parallel, per the dst-ownership sharding):
  - Nodes are sharded contiguously: core c owns nodes [c*NOWN, (c+1)*NOWN).
  - Edges live on the core owning dst; per core they are grouped by
    128-node dst groups and sorted so that edge-softmax segment reductions
    become small dense one-hot matmuls on the tensor engine.
  - Per layer: Phase A computes feat/el/er for owned nodes with one matmul
    against W_aug = [W | W@al_bd | W@ar_bd]; an AllGather publishes
    [feat, el] rows to every core; batched SWDGE dma_gather fetches the
    per-edge rows (feat+el by src from the gathered table, er by local dst);
    exp/leaky-relu run on ACT; out[n] = (sum_e ex_e * feat_src) / sum_e ex_e
    via one-hot matmuls with node-level normalization; ELU and a PE
    transpose produce the next layer's x^T.

All graph-dependent index structures are computed on the host inside
kernel() and shipped as tensor inputs, so one SPMD program serves all
8 cores.
"""

import math
import os

import numpy as np

P = 128
NCORES = 8


# ----------------------------------------------------------------------------
# Host-side preparation
# ----------------------------------------------------------------------------


def _wrap_idxs(idx, k):
    """int16 index array for dma_gather: wrapped in 16 partitions, replicated
    8x across the 128 partitions. idx: [k*128] -> [128, k*8]."""
    assert idx.shape[0] == k * P
    w = idx.astype(np.int16).reshape(k * 8, 16).T  # [16, k*8]
    return np.ascontiguousarray(np.tile(w, (8, 1)))  # [128, k*8]


def _prepare(inputs):
    h = np.asarray(inputs["h"], dtype=np.float32)
    src = np.asarray(inputs["src"]).astype(np.int64)
    dst = np.asarray(inputs["dst"]).astype(np.int64)

    N, NFEAT = h.shape
    E = src.shape[0]
    assert N % NCORES == 0
    NOWN = N // NCORES
    G = math.ceil(NOWN / P)
    HALF = (N + 1) // 2
    assert HALF <= 32767, "table half exceeds int16 gather index range"
    assert NOWN <= 32767

    Ws, als, ars = [], [], []
    for i in (1, 2, 3):
        Ws.append(np.asarray(inputs[f"W{i}"], dtype=np.float32))
        als.append(np.asarray(inputs[f"al{i}"], dtype=np.float32))
        ars.append(np.asarray(inputs[f"ar{i}"], dtype=np.float32))
    H = als[0].shape[0]
    FEAT = [W.shape[1] for W in Ws]  # H*D per layer
    D = [f // H for f in FEAT]
    NCLASS = D[-1]

    # W_aug = [W | W @ al_bd | W @ ar_bd] with al_bd[h*D+d, h] = al[h, d]
    Waug = []
    for W, al, ar, f, d in zip(Ws, als, ars, FEAT, D):
        al_bd = np.zeros((f, H), dtype=np.float32)
        ar_bd = np.zeros((f, H), dtype=np.float32)
        for hh in range(H):
            al_bd[hh * d : (hh + 1) * d, hh] = al[hh]
            ar_bd[hh * d : (hh + 1) * d, hh] = ar[hh]
        Waug.append(
            np.ascontiguousarray(
                np.concatenate([W, W @ al_bd, W @ ar_bd], axis=1), dtype=np.float32
            )
        )
    FO = [f + 2 * H for f in FEAT]
    # gather-table row widths (bytes must be a multiple of 256)
    ROW = [math.ceil((f + H) * 4 / 256) * 64 for f in FEAT]

    # ---- edge partitioning --------------------------------------------------
    owner = dst // NOWN
    per_core = []
    maxA = maxB = 0
    for c in range(NCORES):
        sel = np.nonzero(owner == c)[0]
        e_src = src[sel]
        e_dst = dst[sel]
        dloc = e_dst - c * NOWN  # 0..NOWN-1
        grp = dloc // P  # dst group
        half = (e_src >= HALF).astype(np.int64)
        order = np.lexsort((e_src, half, grp))
        e_src, dloc, grp, half = e_src[order], dloc[order], grp[order], half[order]
        cntA = np.zeros(G, dtype=np.int64)
        cntB = np.zeros(G, dtype=np.int64)
        for g in range(G):
            m = grp == g
            cntA[g] = int(np.count_nonzero(m & (half == 0)))
            cntB[g] = int(np.count_nonzero(m & (half == 1)))
        per_core.append((e_src, dloc, grp, half, cntA, cntB))
        maxA = max(maxA, int(cntA.max()) if G else 0)
        maxB = max(maxB, int(cntB.max()) if G else 0)

    kA = max(1, math.ceil(maxA / P))
    kB = max(1, math.ceil(maxB / P))
    K = kA + kB

    in_maps = []
    for c in range(NCORES):
        e_src, dloc, grp, half, cntA, cntB = per_core[c]
        idxA = np.zeros((G, kA * P), dtype=np.int64)
        idxB = np.zeros((G, kB * P), dtype=np.int64)
        idxE = np.zeros((G, K * P), dtype=np.int64)
        dstf = np.full((G, K * P), -1.0, dtype=np.float32)
        pos = 0
        for g in range(G):
            nA, nB = int(cntA[g]), int(cntB[g])
            sA = e_src[pos : pos + nA]
            dA = dloc[pos : pos + nA]
            sB = e_src[pos + nA : pos + nA + nB] - HALF
            dB = dloc[pos + nA : pos + nA + nB]
            pos += nA + nB
            idxA[g, :nA] = sA
            idxB[g, :nB] = sB
            # er gather indices follow the same slot order (A slots, then B)
            idxE[g, :nA] = dA
            idxE[g, kA * P : kA * P + nB] = dB
            dstf[g, :nA] = (dA - g * P).astype(np.float32)
            dstf[g, kA * P : kA * P + nB] = (dB - g * P).astype(np.float32)

        idxA_sb = np.concatenate([_wrap_idxs(idxA[g], kA) for g in range(G)], axis=1)
        idxB_sb = np.concatenate([_wrap_idxs(idxB[g], kB) for g in range(G)], axis=1)
        idxE_sb = np.concatenate([_wrap_idxs(idxE[g], K) for g in range(G)], axis=1)
        # dstf as SBUF layout [128, G*K]: [p, g*K+t] = dst_local of slot t*128+p
        dstf_sb = np.ascontiguousarray(dstf.reshape(G * K, P).T)

        hT = np.ascontiguousarray(h[c * NOWN : (c + 1) * NOWN, :].T)

        m = {
            "hT": hT,
            "iota": np.broadcast_to(
                np.arange(P, dtype=np.float32)[None, :], (P, P)
            ).copy(),
            "ident": np.eye(P, dtype=np.float32),
            "dstf": dstf_sb,
            "idxA": idxA_sb,
            "idxB": idxB_sb,
            "idxE": idxE_sb,
            "Wa1": Waug[0],
            "Wa2": Waug[1],
            "Wa3": Waug[2],
        }
        in_maps.append(m)

    cfg = dict(
        N=N,
        E=E,
        NFEAT=NFEAT,
        NOWN=NOWN,
        G=G,
        HALF=HALF,
        H=H,
        FEAT=FEAT,
        D=D,
        FO=FO,
        ROW=ROW,
        NCLASS=NCLASS,
        kA=kA,
        kB=kB,
        K=K,
    )
    return cfg, in_maps


# ----------------------------------------------------------------------------
# Bass program
# ----------------------------------------------------------------------------


def _build(cfg, mm_f32r=True):
    import concourse.bacc as bacc
    import concourse.mybir as mybir
    import concourse.tile as tile

    NOWN, G, HALF = cfg["NOWN"], cfg["G"], cfg["HALF"]
    N, NFEAT, H = cfg["N"], cfg["NFEAT"], cfg["H"]
    FEAT, FO, ROW, D = cfg["FEAT"], cfg["FO"], cfg["ROW"], cfg["D"]
    NCLASS = cfg["NCLASS"]
    kA, kB, K = cfg["kA"], cfg["kB"], cfg["K"]
    NEG = 0.2
    f32 = mybir.dt.float32
    f32r = mybir.dt.float32r
    i16 = mybir.dt.int16
    AF = mybir.ActivationFunctionType
    OP = mybir.AluOpType

    F_IN = [NFEAT, FEAT[0], FEAT[1]]
    KT = [math.ceil(f / P) for f in F_IN]
    KTmax = max(KT)

    nc = bacc.Bacc(
        "TRN2", target_bir_lowering=False, debug=False, num_devices=NCORES
    )

    mmdt = f32r if mm_f32r else f32

    # ---- I/O ----------------------------------------------------------------
    hT_d = nc.dram_tensor("hT", [NFEAT, NOWN], mmdt, kind="ExternalInput")
    iota_d = nc.dram_tensor("iota", [P, P], f32, kind="ExternalInput")
    ident_d = nc.dram_tensor("ident", [P, P], f32, kind="ExternalInput")
    dstf_d = nc.dram_tensor("dstf", [P, G * K], f32, kind="ExternalInput")
    idxA_d = nc.dram_tensor("idxA", [P, G * kA * 8], i16, kind="ExternalInput")
    idxB_d = nc.dram_tensor("idxB", [P, G * kB * 8], i16, kind="ExternalInput")
    idxE_d = nc.dram_tensor("idxE", [P, G * K * 8], i16, kind="ExternalInput")
    W_d = [
        nc.dram_tensor(f"Wa{i + 1}", [F_IN[i], FO[i]], mmdt, kind="ExternalInput")
        for i in range(3)
    ]
    out_d = nc.dram_tensor("out", [NOWN, NCLASS], f32, kind="ExternalOutput")

    # internal DRAM per layer
    ag_in = [
        nc.dram_tensor(f"ag_in{i}", [NOWN, ROW[i]], f32, kind="Internal")
        for i in range(3)
    ]
    ag_out = [
        nc.dram_tensor(
            f"ag_out{i}", [NCORES * NOWN, ROW[i]], f32, kind="Internal",
            addr_space="Shared",
        )
        for i in range(3)
    ]
    er_tab = [
        nc.dram_tensor(f"er_tab{i}", [G * P, 64], f32, kind="Internal")
        for i in range(3)
    ]

    rg = [list(range(NCORES))]

    with tile.TileContext(nc, num_cores=NCORES) as tc:
        with (
            tc.tile_pool(name="const", bufs=1) as cpool,
            tc.tile_pool(name="xt", bufs=1) as xtpool,
            tc.tile_pool(name="work", bufs=2) as wpool,
            tc.tile_pool(name="gath", bufs=2) as gpool,
            tc.tile_pool(name="psum", bufs=2, space="PSUM") as pspool,
        ):
            iota_t = cpool.tile([P, P], f32, name="iota_t")
            ident_t = cpool.tile([P, P], f32, name="ident_t")
            dstf_t = cpool.tile([P, G * K], f32, name="dstf_t")
            idxA_t = cpool.tile([P, G * kA * 8], i16, name="idxA_t")
            idxB_t = cpool.tile([P, G * kB * 8], i16, name="idxB_t")
            idxE_t = cpool.tile([P, G * K * 8], i16, name="idxE_t")
            nc.sync.dma_start(iota_t[:], iota_d[:])
            nc.sync.dma_start(ident_t[:], ident_d[:])
            nc.sync.dma_start(dstf_t[:], dstf_d[:])
            nc.sync.dma_start(idxA_t[:], idxA_d[:])
            nc.sync.dma_start(idxB_t[:], idxB_d[:])
            nc.sync.dma_start(idxE_t[:], idxE_d[:])

            W_t = []
            for l in range(3):
                slices = []
                for k in range(KT[l]):
                    r0 = k * P
                    r1 = min(r0 + P, F_IN[l])
                    w = cpool.tile([P, FO[l]], mmdt, name=f"W{l}_{k}")
                    nc.sync.dma_start(w[: r1 - r0, :], W_d[l][r0:r1, :])
                    slices.append(w)
                W_t.append(slices)

            # x^T tiles, [128, NOWN] per 128-row slice of the input features
            xT = [
                cpool.tile([P, NOWN], mmdt, name=f"xT{k}") for k in range(KTmax)
            ]
            for k in range(KT[0]):
                r0, r1 = k * P, min((k + 1) * P, NFEAT)
                nc.sync.dma_start(xT[k][: r1 - r0, :], hT_d[r0:r1, :])

            er_big = cpool.tile([P, G * 64], f32, name="er_big")

            for l in range(3):
                FT, FOL, RW, DL = FEAT[l], FO[l], ROW[l], D[l]
                last = l == 2

                # ---------------- Phase A: feat/el/er for owned nodes -------
                nc.vector.memset(er_big[:], 0.0)
                for g in range(G):
                    nn = min(P, NOWN - g * P)
                    psA = pspool.tile([P, FOL], f32, name="psA", tag="psA")
                    for k in range(KT[l]):
                        kk = min(P, F_IN[l] - k * P)
                        lhs = xT[k][:kk, g * P : g * P + nn]
                        rhs = W_t[l][k][:kk, :]
                        nc.tensor.matmul(
                            psA[:nn, :],
                            lhsT=lhs,
                            rhs=rhs,
                            start=(k == 0),
                            stop=(k == KT[l] - 1),
                        )
                    stage = wpool.tile([P, RW], f32, name="stage", tag="stage")
                    nc.vector.tensor_copy(stage[:nn, 0 : FT + H], psA[:nn, 0 : FT + H])
                    if RW > FT + H:
                        nc.vector.memset(stage[:, FT + H : RW], 0.0)
                    nc.vector.tensor_copy(
                        er_big[:nn, g * 64 : g * 64 + H], psA[:nn, FT + H : FOL]
                    )
                    nc.sync.dma_start(
                        ag_in[l][g * P : g * P + nn, :], stage[:nn, :]
                    )
                # er table: [128, G*64] -> [G*128, 64]
                nc.sync.dma_start(
                    er_tab[l][:].rearrange("(g p) c -> p g c", p=P),
                    er_big[:].rearrange("p (g c) -> p g c", c=64),
                )

                # ---------------- AllGather --------------------------------
                nc.gpsimd.collective_compute(
                    "AllGather",
                    mybir.AluOpType.bypass,
                    replica_groups=rg,
                    ins=[ag_in[l][:]],
                    outs=[ag_out[l][:]],
                )

                tabA = ag_out[l][0:HALF, :]
                tabB = ag_out[l][HALF:N, :]

                # ---------------- Edge phase -------------------------------
                # a dma_gather packs num_idxs/16 + 1 descriptors into one
                # packet per SDMA engine (single_packet); packets are capped
                # at 64 descriptors -> at most 7 tiles (896 idxs) per call.
                GCH = 7

                def emit_gather(dst3, tab, idx_t, idx_col0, ktiles, row):
                    t0 = 0
                    while t0 < ktiles:
                        tc_ = min(GCH, ktiles - t0)
                        nc.gpsimd.dma_gather(
                            dst3[:, t0 : t0 + tc_, :],
                            tab,
                            idx_t[:, idx_col0 + t0 * 8 : idx_col0 + (t0 + tc_) * 8],
                            tc_ * P,
                            tc_ * P,
                            row,
                            elem_step=row,
                        )
                        t0 += tc_

                for g in range(G):
                    nn = min(P, NOWN - g * P)
                    fb = gpool.tile([P, K * RW], f32, name="fb", tag="fb")
                    eb = gpool.tile([P, K * 64], f32, name="eb", tag="eb")
                    f3 = fb[:].rearrange("p (k r) -> p k r", r=RW)
                    e3 = eb[:].rearrange("p (k r) -> p k r", r=64)
                    emit_gather(f3[:, 0:kA, :], tabA, idxA_t, g * kA * 8, kA, RW)
                    emit_gather(f3[:, kA:K, :], tabB, idxB_t, g * kB * 8, kB, RW)
                    emit_gather(e3[:, 0:K, :], er_tab[l][:], idxE_t, g * K * 8, K, 64)

                    # e = exp(leaky_relu(el + er)) for all K tiles at once
                    ee = wpool.tile([P, K * H], f32, name="ee", tag="ee")
                    ee3 = ee[:].rearrange("p (k h) -> p k h", h=H)
                    nc.vector.tensor_add(ee3, f3[:, :, FT : FT + H], e3[:, :, 0:H])
                    # leaky_relu(x) = max(0.2*x, x)
                    nc.vector.scalar_tensor_tensor(
                        out=ee[:], in0=ee[:], scalar=NEG, in1=ee[:],
                        op0=OP.mult, op1=OP.max,
                    )
                    nc.scalar.activation(ee[:], ee[:], AF.Exp)

                    ps_out = pspool.tile([P, FT], f32, name="ps_out", tag="ps_out")
                    ps_s = pspool.tile([P, H], f32, name="ps_s", tag="ps_s", bufs=1)
                    for t in range(K):
                        oh = wpool.tile([P, P], mmdt, name="oh", tag="oh", bufs=3)
                        dcol = dstf_t[:, g * K + t : g * K + t + 1]
                        nc.vector.tensor_tensor(
                            out=oh[:],
                            in0=dcol.to_broadcast([P, P]),
                            in1=iota_t[:],
                            op=OP.is_equal,
                        )
                        fs = wpool.tile([P, FT], mmdt, name="fs", tag="fs", bufs=3)
                        nc.vector.tensor_mul(
                            fs[:].rearrange("p (h d) -> p h d", h=H),
                            f3[:, t, 0:FT].rearrange("p (h d) -> p h d", h=H),
                            ee[:, t * H : (t + 1) * H].to_broadcast([P, H, DL]),
                        )
                        nc.tensor.matmul(
                            ps_out[:],
                            lhsT=oh[:],
                            rhs=fs[:],
                            start=(t == 0),
                            stop=(t == K - 1),
                        )
                        nc.tensor.matmul(
                            ps_s[:],
                            lhsT=oh[:].bitcast(f32),
                            rhs=ee[:, t * H : (t + 1) * H],
                            start=(t == 0),
                            stop=(t == K - 1),
                        )

                    s_r = wpool.tile([P, H], f32, name="s_r", tag="s_r")
                    nc.vector.tensor_scalar_max(s_r[:], ps_s[:], 1e-30)
                    nc.vector.reciprocal(s_r[:], s_r[:])
                    if last:
                        nc.vector.tensor_scalar_mul(s_r[:], s_r[:], 1.0 / H)
                    xg = wpool.tile([P, FT], f32, name="xg", tag="xg")
                    nc.vector.tensor_mul(
                        xg[:].rearrange("p (h d) -> p h d", h=H),
                        ps_out[:].rearrange("p (h d) -> p h d", h=H),
                        s_r[:].to_broadcast([P, H, DL]),
                    )

                    if not last:
                        # elu(x) = max(x, exp(min(x, 0)) - 1), then transpose
                        mg = wpool.tile([P, FT], f32, name="mg", tag="mg")
                        nc.vector.tensor_scalar_min(mg[:], xg[:], 0.0)
                        nc.scalar.activation(mg[:], mg[:], AF.Exp)
                        nc.vector.scalar_tensor_tensor(
                            out=xg[:],
                            in0=mg[:],
                            scalar=-1.0,
                            in1=xg[:],
                            op0=OP.add,
                            op1=OP.max,
                        )
                        for kk in range(KT[l + 1]):
                            c0 = kk * P
                            c1 = min(c0 + P, FT)
                            w = c1 - c0
                            pt = pspool.tile([P, P], f32, name="pt", tag="pt")
                            nc.tensor.transpose(
                                pt[:w, :], xg[:, c0:c1], ident_t[:]
                            )
                            nc.vector.tensor_copy(
                                xT[kk][:w, g * P : g * P + nn], pt[:w, :nn]
                            )
                    else:
                        # mean over heads -> [nn, NCLASS] -> DRAM
                        o1 = wpool.tile([P, NCLASS], f32, name="o1", tag="o1")
                        o2 = wpool.tile([P, NCLASS], f32, name="o2", tag="o2")
                        nc.vector.tensor_add(
                            o1[:], xg[:, 0:NCLASS], xg[:, NCLASS : 2 * NCLASS]
                        )
                        nc.vector.tensor_add(
                            o2[:],
                            xg[:, 2 * NCLASS : 3 * NCLASS],
                            xg[:, 3 * NCLASS : 4 * NCLASS],
                        )
                        nc.vector.tensor_add(o1[:], o1[:], o2[:])
                        nc.sync.dma_start(
                            out_d[g * P : g * P + nn, :], o1[:nn, :]
                        )

    nc.compile()
    return nc


# ----------------------------------------------------------------------------
# Driver
# ----------------------------------------------------------------------------

_CACHE = {}


def _get_nc(cfg, mm_f32r):
    key = (tuple(sorted(cfg.items(), key=lambda kv: kv[0], reverse=False))
           if False else str(sorted(cfg.items())) + str(mm_f32r))
    if key not in _CACHE:
        _CACHE[key] = _build(cfg, mm_f32r=mm_f32r)
    return _CACHE[key]


def _run(inputs, trace=False, mm_f32r=True, use_sim=False, bench_iters=0):
    cfg, in_maps = _prepare(inputs)
    nc = _get_nc(cfg, mm_f32r)
    NOWN, NCLASS = cfg["NOWN"], cfg["NCLASS"]

    if use_sim:
        from concourse.bass_interp import MultiCoreSim

        sim = MultiCoreSim(nc, num_cores=NCORES, require_finite=False)
        for c in range(NCORES):
            for k, v in in_maps[c].items():
                sim.cores[c].tensor(k)[:] = v
        sim.simulate(check_with_hw=False)
        outs = [np.array(sim.cores[c].tensor("out")) for c in range(NCORES)]
        res = None
    else:
        outs, res = _pjrt_run(nc, in_maps, bench_iters=bench_iters)

    out = np.concatenate(outs, axis=0).astype(np.float32)
    return out, res


def _pjrt_run(nc, in_maps, bench_iters=0):
    """Execute the SPMD program on the 8 axon-tunneled cores via PJRT.

    Mirrors concourse.bass2jax.run_bass_via_pjrt but keeps the compiled
    executable so warm re-runs can be timed (bench_iters > 0)."""
    import time as _time

    import jax
    import numpy as _np
    from jax.sharding import Mesh, PartitionSpec
    from jax.experimental.shard_map import shard_map

    import concourse.mybir as mybir
    from concourse.bass2jax import (
        _bass_exec_p,
        install_neuronx_cc_hook,
        partition_id_tensor,
    )

    install_neuronx_cc_hook()
    n_cores = len(in_maps)

    partition_name = nc.partition_id_tensor.name if nc.partition_id_tensor else None
    in_names, out_names, out_avals, zero_outs = [], [], [], []
    for alloc in nc.m.functions[0].allocations:
        if not isinstance(alloc, mybir.MemoryLocationSet):
            continue
        name = alloc.memorylocations[0].name
        if alloc.kind == "ExternalInput":
            if name != partition_name:
                in_names.append(name)
        elif alloc.kind == "ExternalOutput":
            shape = tuple(alloc.tensor_shape)
            dtype = mybir.dt.np(alloc.dtype)
            out_names.append(name)
            out_avals.append(jax.core.ShapedArray(shape, dtype))
            zero_outs.append(_np.zeros(shape, dtype))
    n_params = len(in_names)
    n_outs = len(out_avals)
    in_names_all = list(in_names) + list(out_names)
    if partition_name is not None:
        in_names_all.append(partition_name)
    donate = tuple(range(n_params, n_params + n_outs))

    def _body(*args):
        operands = list(args)
        if partition_name is not None:
            operands.append(partition_id_tensor())
        outs = _bass_exec_p.bind(
            *operands,
            out_avals=tuple(out_avals),
            in_names=tuple(in_names_all),
            out_names=tuple(out_names),
            lowering_input_output_aliases=(),
            sim_require_finite=True,
            sim_require_nnan=True,
            nc=nc,
        )
        return tuple(outs)

    devices = jax.devices()[:n_cores]
    mesh = Mesh(_np.asarray(devices), ("core",))
    in_specs = (PartitionSpec("core"),) * (n_params + n_outs)
    out_specs = (PartitionSpec("core"),) * n_outs
    sharded = jax.jit(
        shard_map(
            _body, mesh=mesh, in_specs=in_specs, out_specs=out_specs,
            check_rep=False,
        ),
        donate_argnums=donate,
        keep_unused=True,
    )
    concat_in = [
        _np.concatenate([_np.asarray(in_maps[c][nm]) for c in range(n_cores)], axis=0)
        for nm in in_names
    ]

    def _zeros_dev():
        return [
            jax.device_put(
                _np.zeros((n_cores * z.shape[0], *z.shape[1:]), z.dtype),
                jax.sharding.NamedSharding(mesh, PartitionSpec("core")),
            )
            for z in zero_outs
        ]

    dev_in = [
        jax.device_put(a, jax.sharding.NamedSharding(mesh, PartitionSpec("core")))
        for a in concat_in
    ]

    out_arrs = sharded(*dev_in, *_zeros_dev())
    jax.block_until_ready(out_arrs)

    times = []
    for _ in range(bench_iters):
        zs = _zeros_dev()
        jax.block_until_ready(zs)
        t0 = _time.perf_counter()
        o = sharded(*dev_in, *zs)
        jax.block_until_ready(o)
        times.append(_time.perf_counter() - t0)

    outs = [
        {
            nm: _np.asarray(out_arrs[i]).reshape(n_cores, *out_avals[i].shape)[c]
            for i, nm in enumerate(out_names)
        }
        for c in range(n_cores)
    ]
    res = {"times_s": times, "min_time_ns": int(min(times) * 1e9) if times else None}
    return [o["out"] for o in outs], res


def kernel(**inputs):
    out, _ = _run(inputs, trace=False)
    return out

